# revision 4
# baseline (speedup 1.0000x reference)
"""MoE MLP (top-2, capacity-dropped) Trainium2 kernel — expert-parallel over 8 cores.

Contract: kernel(**inputs) takes the FULL numpy inputs of nn_MoEMLP_53111565582398
and returns (y [4,2048,1024] f32, aux_loss scalar f32), matching reference().

Per-core program (SPMD, core c owns expert c):
  - replicated fp32 router: logits = x @ router_w + router_b, softmax, top-2
    (selection done on logits; exact-fp32 so token selection matches reference)
  - compaction: per-128-token-tile partition prefix-sum via triangular matmul,
    running cross-tile base kept in an accumulating PSUM; each token assigned
    to this core's expert gets a dense slot in [0, count), others slot=CAP
  - dispatch: indirect row-scatter of x rows into x_slots[CAP,C] (OOB-skip)
  - expert MLP: H split in 2 halves (weights SBUF-resident), token chunks,
    GEMM1 -> gelu(+b1) -> GEMM2 (+b2), y accumulated across halves via
    DMA accum-add into y_slots[CAP+1,C] (row CAP = zero dump row)
  - combine: indirect row-gather y_slots[slot] * gate -> y_partial[S,C]
  - aux loss computed on core 0 (all cores): E * sum(mean_probs * counts/total)
Host: shards/preshapes inputs, sums the 8 y_partial outputs, reshapes.
"""
import sys
import numpy as np

sys.path.insert(0, "/opt/trn_rl_repo")

# ---- problem constants (hardcoded per contract) ----
B, T, C = 4, 2048, 1024
S = B * T                  # 8192
H = 4096
E = 8
TOP_K = 2
CAP = 2560                 # ceil(1.25 * S * 2 / 8)
P = 128
T_TILES = S // P           # 64
N_HALF = 2                 # H split halves (weights resident per half)
H_HALF = H // N_HALF       # 2048
KO1 = C // P               # 8  (C subtiles, GEMM1 contraction)
KO2 = H_HALF // P          # 16 (H-half subtiles, GEMM2 contraction)
M1 = H_HALF // P           # 16 GEMM1 output tiles per half
CHUNK = 256                # token chunk
N_CHUNKS = CAP // CHUNK    # 10
N_CORES = 8

_CACHE = {}


def _build():
    """Build + compile the per-core Bass program (same program for all cores)."""
    if "nc" in _CACHE:
        return _CACHE["nc"]
    import concourse.bass as bass
    import concourse.mybir as mybir
    import concourse.tile as tile
    from concourse import bacc
    from concourse.masks import make_identity

    f32 = mybir.dt.float32
    i32 = mybir.dt.int32
    AF = mybir.ActivationFunctionType
    OP = mybir.AluOpType
    AX = mybir.AxisListType

    nc = bacc.Bacc("TRN2", target_bir_lowering=False, debug=False,
                   num_devices=N_CORES)

    x_in = nc.dram_tensor("x", [S, C], f32, kind="ExternalInput")
    rw_in = nc.dram_tensor("rw", [P, KO1, E], f32, kind="ExternalInput")
    rb_in = nc.dram_tensor("rb", [P, E], f32, kind="ExternalInput")
    oh_in = nc.dram_tensor("oh", [P, E], f32, kind="ExternalInput")
    w1_in = nc.dram_tensor("w1", [C, H], f32, kind="ExternalInput")
    b1_in = nc.dram_tensor("b1p", [P, H // P], f32, kind="ExternalInput")
    w2_in = nc.dram_tensor("w2", [H, C], f32, kind="ExternalInput")
    b2_in = nc.dram_tensor("b2r", [P, C], f32, kind="ExternalInput")

    y_out = nc.dram_tensor("y_partial", [S, C], f32, kind="ExternalOutput")
    aux_out = nc.dram_tensor("aux", [1, 1], f32, kind="ExternalOutput")

    x_slots = nc.dram_tensor("x_slots", [CAP, C], f32)
    y_slots = nc.dram_tensor("y_slots", [CAP + 1, C], f32)

    w1_r = w1_in.ap().rearrange("(ko p) h -> p ko h", p=P)    # [P, 8, H]
    w2_r = w2_in.ap().rearrange("(ko p) c -> p ko c", p=P)    # [P, 32, C]

    with tile.TileContext(nc) as tc:
        with (
            tc.tile_pool(name="const", bufs=1) as const,
            tc.tile_pool(name="wpool", bufs=1) as wpool,
            tc.tile_pool(name="big", bufs=4) as big,       # [P,C] x/xsl/yt/ystage
            tc.tile_pool(name="xt", bufs=1) as xtp,        # xT chunk
            tc.tile_pool(name="ht", bufs=1) as htp,        # hT chunk
            tc.tile_pool(name="small", bufs=6) as small,
            tc.tile_pool(name="keep", bufs=1) as keep,
            tc.tile_pool(name="tp_ps", bufs=2, space="PSUM") as tp_ps,
            tc.tile_pool(name="lg_ps", bufs=2, space="PSUM") as lg_ps,
            tc.tile_pool(name="base_ps", bufs=1, space="PSUM") as base_psp,
            tc.tile_pool(name="h_ps", bufs=2, space="PSUM") as h_ps,
            tc.tile_pool(name="y_ps", bufs=1, space="PSUM") as y_ps,
        ):
            # ---------- constants ----------
            ident = const.tile([P, P], f32)
            make_identity(nc, ident[:])
            u_incl = const.tile([P, P], f32)   # lhsT: u[q,p]=1 iff q<=p
            nc.gpsimd.memset(u_incl[:], 1.0)
            nc.gpsimd.affine_select(out=u_incl[:], in_=u_incl[:],
                                    compare_op=OP.is_ge, fill=0.0, base=0,
                                    pattern=[[1, P]], channel_multiplier=-1)
            ones_mat = const.tile([P, P], f32)
            nc.gpsimd.memset(ones_mat[:], 1.0)

            rw_sb = const.tile([P, KO1, E], f32)
            nc.sync.dma_start(rw_sb[:], rw_in[:])
            rb_sb = const.tile([P, E], f32)
            nc.sync.dma_start(rb_sb[:], rb_in[:])
            oh_sb = const.tile([P, E], f32)
            nc.sync.dma_start(oh_sb[:], oh_in[:])
            b1_sb = const.tile([P, H // P], f32)
            nc.sync.dma_start(b1_sb[:], b1_in[:])
            b2_sb = const.tile([P, C], f32)
            nc.sync.dma_start(b2_sb[:], b2_in[:])

            # zero dump row of y_slots (gathered by unassigned tokens)
            zrow = const.tile([1, C], f32)
            nc.vector.memset(zrow[:], 0.0)
            nc.sync.dma_start(y_slots[CAP:CAP + 1, :], zrow[:])

            # ---------- persistent routing state ----------
            gate_mine = keep.tile([P, T_TILES], f32)
            slot_i = keep.tile([P, T_TILES], i32)
            sum_probs = keep.tile([P, E], f32)
            cnt_all = keep.tile([P, E], f32)
            nc.vector.memset(sum_probs[:], 0.0)
            nc.vector.memset(cnt_all[:], 0.0)
            base_ps = base_psp.tile([P, 1], f32)

            # ---------- phase 1: router + dispatch ----------
            for t in range(T_TILES):
                x_tile = big.tile([P, C], f32, tag="bigtile")
                nc.sync.dma_start(x_tile[:], x_in[t * P:(t + 1) * P, :])

                xT = xtp.tile([P, KO1, CHUNK], f32, tag="xT")  # use [:, :, :P]
                for k in range(KO1):
                    tp = tp_ps.tile([P, P], f32, tag="tp")
                    nc.tensor.transpose(tp[:], x_tile[:, k * P:(k + 1) * P],
                                        ident[:])
                    nc.any.tensor_copy(xT[:, k, 0:P], tp[:])

                lg = lg_ps.tile([P, E], f32, tag="lg")
                for k in range(KO1):
                    nc.tensor.matmul(lg[:], lhsT=xT[:, k, 0:P],
                                     rhs=rw_sb[:, k, :],
                                     start=(k == 0), stop=(k == KO1 - 1))
                logits = small.tile([P, E], f32, tag="logits")
                nc.vector.tensor_tensor(logits[:], lg[:], rb_sb[:], op=OP.add)

                # softmax (probs) — values only; selection uses logits
                mx = small.tile([P, 1], f32, tag="mx")
                nc.vector.tensor_reduce(mx[:], logits[:], axis=AX.X, op=OP.max)
                mxn = small.tile([P, 1], f32, tag="mxn")
                nc.vector.tensor_scalar_mul(mxn[:], mx[:], -1.0)
                ex = small.tile([P, E], f32, tag="ex")
                sume = small.tile([P, 1], f32, tag="sume")
                nc.scalar.activation(ex[:], logits[:], AF.Exp,
                                     bias=mxn[:, 0:1], scale=1.0,
                                     accum_out=sume[:, 0:1])
                rsum = small.tile([P, 1], f32, tag="rsum")
                nc.vector.reciprocal(rsum[:], sume[:])
                probs = small.tile([P, E], f32, tag="probs")
                nc.vector.tensor_scalar_mul(probs[:], ex[:], rsum[:, 0:1])

                # top-2 mask on logits
                m8 = small.tile([P, 8], f32, tag="m8")
                nc.vector.max(m8[:], logits[:])
                mask = small.tile([P, E], f32, tag="mask")
                nc.vector.tensor_scalar(mask[:], logits[:], m8[:, 1:2], None,
                                        op0=OP.is_ge)

                # my expert's gate
                gm_t = small.tile([P, E], f32, tag="gm_t")
                nc.vector.tensor_tensor(gm_t[:], probs[:], oh_sb[:], op=OP.mult)
                nc.vector.tensor_tensor(gm_t[:], gm_t[:], mask[:], op=OP.mult)
                nc.vector.tensor_reduce(gate_mine[:, t:t + 1], gm_t[:],
                                        axis=AX.X, op=OP.add)
                mask_mine = small.tile([P, 1], f32, tag="mask_mine")
                nc.vector.tensor_scalar(mask_mine[:], gate_mine[:, t:t + 1],
                                        0.0, None, op0=OP.is_gt)

                # aux-loss accumulators
                nc.vector.tensor_tensor(sum_probs[:], sum_probs[:], probs[:],
                                        op=OP.add)
                nc.vector.tensor_tensor(cnt_all[:], cnt_all[:], mask[:],
                                        op=OP.add)

                # slot assignment: pos = excl-cumsum within tile + base
                incl = lg_ps.tile([P, E], f32, tag="lg")
                nc.tensor.matmul(incl[:, 0:1], lhsT=u_incl[:], rhs=mask_mine[:],
                                 start=True, stop=True)
                pos = small.tile([P, 1], f32, tag="pos")
                if t == 0:
                    nc.vector.tensor_tensor(pos[:], incl[:, 0:1], mask_mine[:],
                                            op=OP.subtract)
                else:
                    nc.vector.scalar_tensor_tensor(
                        out=pos[:], in0=incl[:, 0:1], scalar=base_ps[:, 0:1],
                        in1=mask_mine[:], op0=OP.add, op1=OP.subtract)
                nc.tensor.matmul(base_ps[:], lhsT=ones_mat[:], rhs=mask_mine[:],
                                 start=(t == 0), stop=(t == T_TILES - 1),
                                 skip_group_check=True)

                slot_f = small.tile([P, 1], f32, tag="slot_f")
                nc.vector.scalar_tensor_tensor(
                    out=slot_f[:], in0=pos[:], scalar=-float(CAP),
                    in1=mask_mine[:], op0=OP.add, op1=OP.mult)
                nc.vector.tensor_scalar_add(slot_f[:], slot_f[:], float(CAP))
                nc.vector.tensor_copy(slot_i[:, t:t + 1], slot_f[:])

                # dispatch: scatter kept x rows to x_slots[slot]
                nc.gpsimd.indirect_dma_start(
                    out=x_slots[:], out_offset=bass.IndirectOffsetOnAxis(
                        ap=slot_i[:, t:t + 1], axis=0),
                    in_=x_tile[:], in_offset=None,
                    bounds_check=CAP - 1, oob_is_err=False)

            # ---------- phase 2: aux loss ----------
            cnt_row = lg_ps.tile([P, E], f32, tag="lg")
            nc.tensor.matmul(cnt_row[0:1, :], lhsT=ones_mat[:, 0:1],
                             rhs=cnt_all[:], start=True, stop=True)
            sp_row = lg_ps.tile([P, E], f32, tag="lg")
            nc.tensor.matmul(sp_row[0:1, :], lhsT=ones_mat[:, 0:1],
                             rhs=sum_probs[:], start=True, stop=True)
            counts = small.tile([1, E], f32, tag="aux_c")
            nc.vector.tensor_copy(counts[:], cnt_row[0:1, :])
            sprobs = small.tile([1, E], f32, tag="aux_s")
            nc.vector.tensor_copy(sprobs[:], sp_row[0:1, :])
            total = small.tile([1, 1], f32, tag="aux_t")
            nc.vector.tensor_reduce(total[:], counts[:], axis=AX.X, op=OP.add)
            nc.vector.tensor_scalar_add(total[:], total[:], 1e-9)
            rec = small.tile([1, 1], f32, tag="aux_r")
            nc.vector.reciprocal(rec[:], total[:])
            prod = small.tile([1, E], f32, tag="aux_p")
            nc.vector.tensor_tensor(prod[:], counts[:], sprobs[:], op=OP.mult)
            dot = small.tile([1, 1], f32, tag="aux_d")
            nc.vector.tensor_reduce(dot[:], prod[:], axis=AX.X, op=OP.add)
            nc.vector.tensor_tensor(dot[:], dot[:], rec[:], op=OP.mult)
            nc.vector.tensor_scalar_mul(dot[:], dot[:], float(E) / float(S))
            nc.sync.dma_start(aux_out[:], dot[:])

            # ---------- phase 3: expert MLP over halves / token chunks ----------
            for half in range(N_HALF):
                w1h = wpool.tile([P, KO1, H_HALF], f32, tag="w1h")
                for k in range(KO1):
                    nc.sync.dma_start(
                        w1h[:, k, :],
                        w1_r[:, k, half * H_HALF:(half + 1) * H_HALF])
                w2h = wpool.tile([P, KO2, C], f32, tag="w2h")
                for k in range(KO2):
                    nc.sync.dma_start(w2h[:, k, :], w2_r[:, half * KO2 + k, :])

                for ch in range(N_CHUNKS):
                    # gather chunk rows (direct reads) + transpose -> xTc
                    xTc = xtp.tile([P, KO1, CHUNK], f32, tag="xT")
                    for j in range(CHUNK // P):
                        xsl = big.tile([P, C], f32, tag="bigtile")
                        r0 = ch * CHUNK + j * P
                        nc.sync.dma_start(xsl[:], x_slots[r0:r0 + P, :])
                        for k in range(KO1):
                            tp = tp_ps.tile([P, P], f32, tag="tp")
                            nc.tensor.transpose(
                                tp[:], xsl[:, k * P:(k + 1) * P], ident[:])
                            nc.any.tensor_copy(xTc[:, k, j * P:(j + 1) * P],
                                               tp[:])

                    # GEMM1 + gelu -> hT
                    hT = htp.tile([P, KO2, CHUNK], f32, tag="hT")
                    for m in range(M1):
                        ph = h_ps.tile([P, CHUNK], f32, tag="ph")
                        for k in range(KO1):
                            nc.tensor.matmul(
                                ph[:], lhsT=w1h[:, k, m * P:(m + 1) * P],
                                rhs=xTc[:, k, :],
                                start=(k == 0), stop=(k == KO1 - 1))
                        nc.scalar.activation(
                            hT[:, m, :], ph[:], AF.Gelu,
                            bias=b1_sb[:, half * M1 + m:half * M1 + m + 1],
                            scale=1.0)

                    # GEMM2 (+b2 on half 0) -> y_slots (add on half 1)
                    for mt in range(CHUNK // P):
                        ystage = big.tile([P, C], f32, tag="bigtile")
                        for n2 in range(C // 512):
                            py = y_ps.tile([P, 512], f32, tag="py")
                            for k in range(KO2):
                                nc.tensor.matmul(
                                    py[:], lhsT=hT[:, k, mt * P:(mt + 1) * P],
                                    rhs=w2h[:, k, n2 * 512:(n2 + 1) * 512],
                                    start=(k == 0), stop=(k == KO2 - 1))
                            if half == 0:
                                nc.vector.tensor_tensor(
                                    ystage[:, n2 * 512:(n2 + 1) * 512], py[:],
                                    b2_sb[:, n2 * 512:(n2 + 1) * 512],
                                    op=OP.add)
                            else:
                                nc.any.tensor_copy(
                                    ystage[:, n2 * 512:(n2 + 1) * 512], py[:])
                        r0 = ch * CHUNK + mt * P
                        if half == 0:
                            nc.sync.dma_start(y_slots[r0:r0 + P, :], ystage[:])
                        else:
                            nc.gpsimd.dma_start(y_slots[r0:r0 + P, :],
                                                ystage[:],
                                                accum_op=OP.add)

            # ---------- phase 4: combine ----------
            for t in range(T_TILES):
                yt = big.tile([P, C], f32, tag="bigtile")
                nc.gpsimd.indirect_dma_start(
                    out=yt[:], out_offset=None,
                    in_=y_slots[:], in_offset=bass.IndirectOffsetOnAxis(
                        ap=slot_i[:, t:t + 1], axis=0),
                    bounds_check=CAP, oob_is_err=False)
                nc.vector.tensor_scalar_mul(yt[:], yt[:],
                                            gate_mine[:, t:t + 1])
                nc.sync.dma_start(y_out[t * P:(t + 1) * P, :], yt[:])

    nc.compile()
    _CACHE["nc"] = nc
    return nc


def _shard_inputs(x, router_w, router_b, w1, b1, w2, b2):
    """Host-side shard/preshape. Returns in_maps for the 8 cores."""
    x_flat = np.ascontiguousarray(np.asarray(x, np.float32).reshape(S, C))
    rw = np.asarray(router_w, np.float32)
    rw_p = np.ascontiguousarray(rw.reshape(KO1, P, E).transpose(1, 0, 2))
    rb_r = np.broadcast_to(np.asarray(router_b, np.float32)[None, :],
                           (P, E)).copy()
    in_maps = []
    for c in range(N_CORES):
        oh = np.zeros((P, E), np.float32)
        oh[:, c] = 1.0
        b1p = np.ascontiguousarray(
            np.asarray(b1[c], np.float32).reshape(H // P, P).T)
        b2r = np.broadcast_to(np.asarray(b2[c], np.float32)[None, :],
                              (P, C)).copy()
        in_maps.append({
            "x": x_flat,
            "rw": rw_p,
            "rb": rb_r,
            "oh": oh,
            "w1": np.ascontiguousarray(np.asarray(w1[c], np.float32)),
            "b1p": b1p,
            "w2": np.ascontiguousarray(np.asarray(w2[c], np.float32)),
            "b2r": b2r,
        })
    return in_maps


def _install_ntff_hook():
    """Register the axon NTFF profile hook (missing antenv.axon_hooks shim)."""
    import types
    if "antenv.axon_hooks" in sys.modules:
        return True
    try:
        from trn_agent_boot import trn_boot
    except ImportError:
        return False
    mod = types.ModuleType("antenv.axon_hooks")
    holder = {}
    mod.set_axon_ntff_profile_hook = lambda h: holder.__setitem__("h", h)
    mod.get_axon_ntff_profile_hook = lambda: holder.get("h")
    sys.modules["antenv.axon_hooks"] = mod
    try:
        import antenv
        antenv.axon_hooks = mod
    except ImportError:
        pass
    try:
        mod.set_axon_ntff_profile_hook(
            trn_boot._ntff_profile_via_ctypes("/opt/axon/libaxon_pjrt.so"))
    except Exception:
        return False
    return True


def kernel(x, router_w, router_b, w1, b1, w2, b2, _trace=False):
    from concourse.bass_utils import run_bass_kernel_spmd

    if _trace:
        _trace = _install_ntff_hook()

    nc = _build()
    in_maps = _shard_inputs(x, router_w, router_b, w1, b1, w2, b2)
    res = run_bass_kernel_spmd(nc, in_maps, core_ids=list(range(N_CORES)),
                               trace=_trace)
    y = np.zeros((S, C), np.float32)
    for c in range(N_CORES):
        y += res.results[c]["y_partial"]
    aux = np.float32(res.results[0]["aux"][0, 0])
    if _trace:
        kernel.last_exec_time_ns = res.exec_time_ns
    return y.reshape(B, T, C), aux


# revision 6
# speedup vs baseline: 2.0646x; 2.0646x over previous
"""MoE MLP (top-2, capacity-dropped) Trainium2 kernel — expert-parallel over 8 cores.

Contract: kernel(**inputs) takes the FULL numpy inputs of nn_MoEMLP_53111565582398
and returns (y [4,2048,1024] f32, aux_loss scalar f32), matching reference().

Per-core program (SPMD, core c owns expert c):
  - replicated fp32 router: logits = x @ router_w + router_b, softmax, top-2
    (selection done on logits; exact-fp32 so token selection matches reference)
  - compaction: per-128-token-tile partition prefix-sum via triangular matmul,
    running cross-tile base kept in an accumulating PSUM; each token assigned
    to this core's expert gets a dense slot in [0, count), others slot=CAP
  - dispatch: indirect row-scatter of x rows into x_slots[CAP,C] (OOB-skip)
  - expert MLP: H split in 2 halves (weights SBUF-resident), token chunks,
    GEMM1 -> gelu(+b1) -> GEMM2 (+b2), y accumulated across halves via
    DMA accum-add into y_slots[CAP+1,C] (row CAP = zero dump row)
  - combine: indirect row-gather y_slots[slot] * gate -> y_partial[S,C]
  - aux loss computed on core 0 (all cores): E * sum(mean_probs * counts/total)
Host: shards/preshapes inputs, sums the 8 y_partial outputs, reshapes.
"""
import sys
import numpy as np

sys.path.insert(0, "/opt/trn_rl_repo")

# ---- problem constants (hardcoded per contract) ----
B, T, C = 4, 2048, 1024
S = B * T                  # 8192
H = 4096
E = 8
TOP_K = 2
CAP = 2560                 # ceil(1.25 * S * 2 / 8)
P = 128
T_TILES = S // P           # 64
N_HALF = 2                 # H split halves (weights resident per half)
H_HALF = H // N_HALF       # 2048
KO1 = C // P               # 8  (C subtiles, GEMM1 contraction)
KO2 = H_HALF // P          # 16 (H-half subtiles, GEMM2 contraction)
M1 = H_HALF // P           # 16 GEMM1 output tiles per half
CHUNK = 256                # token chunk
N_CHUNKS = CAP // CHUNK    # 10
N_CORES = 8
GEMM_DTYPE = "f32r"   # "f32" (exact, ~3.3ms) or "f32r" (tf32-like, faster)

_CACHE = {}


def _build():
    """Build + compile the per-core Bass program (same program for all cores)."""
    if "nc" in _CACHE:
        return _CACHE["nc"]
    import concourse.bass as bass
    import concourse.mybir as mybir
    import concourse.tile as tile
    from concourse import bacc
    from concourse.masks import make_identity

    f32 = mybir.dt.float32
    f32g = mybir.dt.float32r if GEMM_DTYPE == "f32r" else mybir.dt.float32
    i32 = mybir.dt.int32
    AF = mybir.ActivationFunctionType
    OP = mybir.AluOpType
    AX = mybir.AxisListType

    nc = bacc.Bacc("TRN2", target_bir_lowering=False, debug=False,
                   num_devices=N_CORES)

    x_in = nc.dram_tensor("x", [S, C], f32, kind="ExternalInput")
    rw_in = nc.dram_tensor("rw", [P, KO1, E], f32, kind="ExternalInput")
    rb_in = nc.dram_tensor("rb", [P, E], f32, kind="ExternalInput")
    oh_in = nc.dram_tensor("oh", [P, E], f32, kind="ExternalInput")
    w1_in = nc.dram_tensor("w1", [C, H], f32g, kind="ExternalInput")
    b1_in = nc.dram_tensor("b1p", [P, H // P], f32, kind="ExternalInput")
    w2_in = nc.dram_tensor("w2", [H, C], f32g, kind="ExternalInput")
    b2_in = nc.dram_tensor("b2r", [P, C], f32, kind="ExternalInput")

    y_out = nc.dram_tensor("y_partial", [S, C], f32, kind="ExternalOutput")
    aux_out = nc.dram_tensor("aux", [1, 1], f32, kind="ExternalOutput")

    x_slots = nc.dram_tensor("x_slots", [CAP, C], f32)
    y_slots = nc.dram_tensor("y_slots", [CAP + 1, C], f32)

    w1_r = w1_in.ap().rearrange("(ko p) h -> p ko h", p=P)    # [P, 8, H]
    w2_r = w2_in.ap().rearrange("(ko p) c -> p ko c", p=P)    # [P, 32, C]

    with tile.TileContext(nc) as tc:
        with (
            tc.tile_pool(name="const", bufs=1) as const,
            tc.tile_pool(name="wpool", bufs=1) as wpool,
            tc.tile_pool(name="big", bufs=4) as big,       # [P,C] x/xsl/yt/ystage
            tc.tile_pool(name="xt", bufs=1) as xtp,        # xT chunk
            tc.tile_pool(name="ht", bufs=1) as htp,        # hT chunk
            tc.tile_pool(name="small", bufs=6) as small,
            tc.tile_pool(name="keep", bufs=1) as keep,
            tc.tile_pool(name="tp_ps", bufs=2, space="PSUM") as tp_ps,
            tc.tile_pool(name="lg_ps", bufs=2, space="PSUM") as lg_ps,
            tc.tile_pool(name="base_ps", bufs=1, space="PSUM") as base_psp,
            tc.tile_pool(name="h_ps", bufs=2, space="PSUM") as h_ps,
            tc.tile_pool(name="y_ps", bufs=1, space="PSUM") as y_ps,
        ):
            # ---------- constants ----------
            ident = const.tile([P, P], f32)
            make_identity(nc, ident[:])
            u_incl = const.tile([P, P], f32)   # lhsT: u[q,p]=1 iff q<=p
            nc.gpsimd.memset(u_incl[:], 1.0)
            nc.gpsimd.affine_select(out=u_incl[:], in_=u_incl[:],
                                    compare_op=OP.is_ge, fill=0.0, base=0,
                                    pattern=[[1, P]], channel_multiplier=-1)
            ones_mat = const.tile([P, P], f32)
            nc.gpsimd.memset(ones_mat[:], 1.0)

            rw_sb = const.tile([P, KO1, E], f32)
            nc.sync.dma_start(rw_sb[:], rw_in[:])
            rb_sb = const.tile([P, E], f32)
            nc.sync.dma_start(rb_sb[:], rb_in[:])
            oh_sb = const.tile([P, E], f32)
            nc.sync.dma_start(oh_sb[:], oh_in[:])
            b1_sb = const.tile([P, H // P], f32)
            nc.sync.dma_start(b1_sb[:], b1_in[:])
            b2_sb = const.tile([P, C], f32)
            nc.sync.dma_start(b2_sb[:], b2_in[:])

            # zero dump row of y_slots (gathered by unassigned tokens)
            zrow = const.tile([1, C], f32)
            nc.vector.memset(zrow[:], 0.0)
            nc.sync.dma_start(y_slots[CAP:CAP + 1, :], zrow[:])

            # ---------- persistent routing state ----------
            gate_mine = keep.tile([P, T_TILES], f32)
            slot_i = keep.tile([P, T_TILES], i32)
            sum_probs = keep.tile([P, E], f32)
            cnt_all = keep.tile([P, E], f32)
            nc.vector.memset(sum_probs[:], 0.0)
            nc.vector.memset(cnt_all[:], 0.0)
            base_ps = base_psp.tile([P, 1], f32)

            # ---------- phase 1: router + dispatch ----------
            for t in range(T_TILES):
                x_tile = big.tile([P, C], f32, tag="bigtile")
                nc.sync.dma_start(x_tile[:], x_in[t * P:(t + 1) * P, :])

                xT = xtp.tile([P, KO1, P], f32, tag="xTr")  # router, exact f32
                for k in range(KO1):
                    tp = tp_ps.tile([P, P], f32, tag="tp")
                    nc.tensor.transpose(tp[:], x_tile[:, k * P:(k + 1) * P],
                                        ident[:])
                    nc.any.tensor_copy(xT[:, k, :], tp[:])

                lg = lg_ps.tile([P, E], f32, tag="lg")
                for k in range(KO1):
                    nc.tensor.matmul(lg[:], lhsT=xT[:, k, :],
                                     rhs=rw_sb[:, k, :],
                                     start=(k == 0), stop=(k == KO1 - 1))
                logits = small.tile([P, E], f32, tag="logits")
                nc.vector.tensor_tensor(logits[:], lg[:], rb_sb[:], op=OP.add)

                # softmax (probs) — values only; selection uses logits
                mx = small.tile([P, 1], f32, tag="mx")
                nc.vector.tensor_reduce(mx[:], logits[:], axis=AX.X, op=OP.max)
                mxn = small.tile([P, 1], f32, tag="mxn")
                nc.vector.tensor_scalar_mul(mxn[:], mx[:], -1.0)
                ex = small.tile([P, E], f32, tag="ex")
                sume = small.tile([P, 1], f32, tag="sume")
                nc.scalar.activation(ex[:], logits[:], AF.Exp,
                                     bias=mxn[:, 0:1], scale=1.0,
                                     accum_out=sume[:, 0:1])
                rsum = small.tile([P, 1], f32, tag="rsum")
                nc.vector.reciprocal(rsum[:], sume[:])
                probs = small.tile([P, E], f32, tag="probs")
                nc.vector.tensor_scalar_mul(probs[:], ex[:], rsum[:, 0:1])

                # top-2 mask on logits
                m8 = small.tile([P, 8], f32, tag="m8")
                nc.vector.max(m8[:], logits[:])
                mask = small.tile([P, E], f32, tag="mask")
                nc.vector.tensor_scalar(mask[:], logits[:], m8[:, 1:2], None,
                                        op0=OP.is_ge)

                # my expert's gate
                gm_t = small.tile([P, E], f32, tag="gm_t")
                nc.vector.tensor_tensor(gm_t[:], probs[:], oh_sb[:], op=OP.mult)
                nc.vector.tensor_tensor(gm_t[:], gm_t[:], mask[:], op=OP.mult)
                nc.vector.tensor_reduce(gate_mine[:, t:t + 1], gm_t[:],
                                        axis=AX.X, op=OP.add)
                mask_mine = small.tile([P, 1], f32, tag="mask_mine")
                nc.vector.tensor_scalar(mask_mine[:], gate_mine[:, t:t + 1],
                                        0.0, None, op0=OP.is_gt)

                # aux-loss accumulators
                nc.vector.tensor_tensor(sum_probs[:], sum_probs[:], probs[:],
                                        op=OP.add)
                nc.vector.tensor_tensor(cnt_all[:], cnt_all[:], mask[:],
                                        op=OP.add)

                # slot assignment: pos = excl-cumsum within tile + base
                incl = lg_ps.tile([P, E], f32, tag="lg")
                nc.tensor.matmul(incl[:, 0:1], lhsT=u_incl[:], rhs=mask_mine[:],
                                 start=True, stop=True)
                pos = small.tile([P, 1], f32, tag="pos")
                if t == 0:
                    nc.vector.tensor_tensor(pos[:], incl[:, 0:1], mask_mine[:],
                                            op=OP.subtract)
                else:
                    nc.vector.scalar_tensor_tensor(
                        out=pos[:], in0=incl[:, 0:1], scalar=base_ps[:, 0:1],
                        in1=mask_mine[:], op0=OP.add, op1=OP.subtract)
                nc.tensor.matmul(base_ps[:], lhsT=ones_mat[:], rhs=mask_mine[:],
                                 start=(t == 0), stop=(t == T_TILES - 1),
                                 skip_group_check=True)

                slot_f = small.tile([P, 1], f32, tag="slot_f")
                nc.vector.scalar_tensor_tensor(
                    out=slot_f[:], in0=pos[:], scalar=-float(CAP),
                    in1=mask_mine[:], op0=OP.add, op1=OP.mult)
                nc.vector.tensor_scalar_add(slot_f[:], slot_f[:], float(CAP))
                nc.vector.tensor_copy(slot_i[:, t:t + 1], slot_f[:])

                # dispatch: scatter kept x rows to x_slots[slot]
                nc.gpsimd.indirect_dma_start(
                    out=x_slots[:], out_offset=bass.IndirectOffsetOnAxis(
                        ap=slot_i[:, t:t + 1], axis=0),
                    in_=x_tile[:], in_offset=None,
                    bounds_check=CAP - 1, oob_is_err=False)

            # ---------- phase 2: aux loss ----------
            cnt_row = lg_ps.tile([P, E], f32, tag="lg")
            nc.tensor.matmul(cnt_row[0:1, :], lhsT=ones_mat[:, 0:1],
                             rhs=cnt_all[:], start=True, stop=True)
            sp_row = lg_ps.tile([P, E], f32, tag="lg")
            nc.tensor.matmul(sp_row[0:1, :], lhsT=ones_mat[:, 0:1],
                             rhs=sum_probs[:], start=True, stop=True)
            counts = small.tile([1, E], f32, tag="aux_c")
            nc.vector.tensor_copy(counts[:], cnt_row[0:1, :])
            sprobs = small.tile([1, E], f32, tag="aux_s")
            nc.vector.tensor_copy(sprobs[:], sp_row[0:1, :])
            total = small.tile([1, 1], f32, tag="aux_t")
            nc.vector.tensor_reduce(total[:], counts[:], axis=AX.X, op=OP.add)
            nc.vector.tensor_scalar_add(total[:], total[:], 1e-9)
            rec = small.tile([1, 1], f32, tag="aux_r")
            nc.vector.reciprocal(rec[:], total[:])
            prod = small.tile([1, E], f32, tag="aux_p")
            nc.vector.tensor_tensor(prod[:], counts[:], sprobs[:], op=OP.mult)
            dot = small.tile([1, 1], f32, tag="aux_d")
            nc.vector.tensor_reduce(dot[:], prod[:], axis=AX.X, op=OP.add)
            nc.vector.tensor_tensor(dot[:], dot[:], rec[:], op=OP.mult)
            nc.vector.tensor_scalar_mul(dot[:], dot[:], float(E) / float(S))
            nc.sync.dma_start(aux_out[:], dot[:])

            # ---------- phase 3: expert MLP over halves / token chunks ----------
            for half in range(N_HALF):
                w1h = wpool.tile([P, KO1, H_HALF], f32g, tag="w1h")
                for k in range(KO1):
                    nc.sync.dma_start(
                        w1h[:, k, :],
                        w1_r[:, k, half * H_HALF:(half + 1) * H_HALF])
                w2h = wpool.tile([P, KO2, C], f32g, tag="w2h")
                for k in range(KO2):
                    nc.sync.dma_start(w2h[:, k, :], w2_r[:, half * KO2 + k, :])

                for ch in range(N_CHUNKS):
                    # gather chunk rows (direct reads) + transpose -> xTc
                    xTc = xtp.tile([P, KO1, CHUNK], f32g, tag="xT")
                    for j in range(CHUNK // P):
                        xsl = big.tile([P, C], f32, tag="bigtile")
                        r0 = ch * CHUNK + j * P
                        nc.sync.dma_start(xsl[:], x_slots[r0:r0 + P, :])
                        for k in range(KO1):
                            tp = tp_ps.tile([P, P], f32, tag="tp")
                            nc.tensor.transpose(
                                tp[:], xsl[:, k * P:(k + 1) * P], ident[:])
                            nc.any.tensor_copy(xTc[:, k, j * P:(j + 1) * P],
                                               tp[:])

                    # GEMM1 + gelu -> hT
                    hT = htp.tile([P, KO2, CHUNK], f32g, tag="hT")
                    for m in range(M1):
                        ph = h_ps.tile([P, CHUNK], f32, tag="ph")
                        for k in range(KO1):
                            nc.tensor.matmul(
                                ph[:], lhsT=w1h[:, k, m * P:(m + 1) * P],
                                rhs=xTc[:, k, :],
                                start=(k == 0), stop=(k == KO1 - 1))
                        nc.scalar.activation(
                            hT[:, m, :], ph[:], AF.Gelu,
                            bias=b1_sb[:, half * M1 + m:half * M1 + m + 1],
                            scale=1.0)

                    # GEMM2 (+b2 on half 0) -> y_slots (add on half 1)
                    for mt in range(CHUNK // P):
                        ystage = big.tile([P, C], f32, tag="bigtile")
                        for n2 in range(C // 512):
                            py = y_ps.tile([P, 512], f32, tag="py")
                            for k in range(KO2):
                                nc.tensor.matmul(
                                    py[:], lhsT=hT[:, k, mt * P:(mt + 1) * P],
                                    rhs=w2h[:, k, n2 * 512:(n2 + 1) * 512],
                                    start=(k == 0), stop=(k == KO2 - 1))
                            if half == 0:
                                nc.vector.tensor_tensor(
                                    ystage[:, n2 * 512:(n2 + 1) * 512], py[:],
                                    b2_sb[:, n2 * 512:(n2 + 1) * 512],
                                    op=OP.add)
                            else:
                                nc.any.tensor_copy(
                                    ystage[:, n2 * 512:(n2 + 1) * 512], py[:])
                        r0 = ch * CHUNK + mt * P
                        if half == 0:
                            nc.sync.dma_start(y_slots[r0:r0 + P, :], ystage[:])
                        else:
                            nc.gpsimd.dma_start(y_slots[r0:r0 + P, :],
                                                ystage[:],
                                                accum_op=OP.add)

            # ---------- phase 4: combine ----------
            for t in range(T_TILES):
                yt = big.tile([P, C], f32, tag="bigtile")
                nc.gpsimd.indirect_dma_start(
                    out=yt[:], out_offset=None,
                    in_=y_slots[:], in_offset=bass.IndirectOffsetOnAxis(
                        ap=slot_i[:, t:t + 1], axis=0),
                    bounds_check=CAP, oob_is_err=False)
                nc.vector.tensor_scalar_mul(yt[:], yt[:],
                                            gate_mine[:, t:t + 1])
                nc.sync.dma_start(y_out[t * P:(t + 1) * P, :], yt[:])

    nc.compile()
    _CACHE["nc"] = nc
    return nc


def _shard_inputs(x, router_w, router_b, w1, b1, w2, b2):
    """Host-side shard/preshape. Returns in_maps for the 8 cores."""
    x_flat = np.ascontiguousarray(np.asarray(x, np.float32).reshape(S, C))
    rw = np.asarray(router_w, np.float32)
    rw_p = np.ascontiguousarray(rw.reshape(KO1, P, E).transpose(1, 0, 2))
    rb_r = np.broadcast_to(np.asarray(router_b, np.float32)[None, :],
                           (P, E)).copy()
    in_maps = []
    for c in range(N_CORES):
        oh = np.zeros((P, E), np.float32)
        oh[:, c] = 1.0
        b1p = np.ascontiguousarray(
            np.asarray(b1[c], np.float32).reshape(H // P, P).T)
        b2r = np.broadcast_to(np.asarray(b2[c], np.float32)[None, :],
                              (P, C)).copy()
        in_maps.append({
            "x": x_flat,
            "rw": rw_p,
            "rb": rb_r,
            "oh": oh,
            "w1": np.ascontiguousarray(np.asarray(w1[c], np.float32)),
            "b1p": b1p,
            "w2": np.ascontiguousarray(np.asarray(w2[c], np.float32)),
            "b2r": b2r,
        })
    return in_maps


def _install_ntff_hook():
    """Register the axon NTFF profile hook (missing antenv.axon_hooks shim)."""
    import types
    if "antenv.axon_hooks" in sys.modules:
        return True
    try:
        from trn_agent_boot import trn_boot
    except ImportError:
        return False
    mod = types.ModuleType("antenv.axon_hooks")
    holder = {}
    mod.set_axon_ntff_profile_hook = lambda h: holder.__setitem__("h", h)
    mod.get_axon_ntff_profile_hook = lambda: holder.get("h")
    sys.modules["antenv.axon_hooks"] = mod
    try:
        import antenv
        antenv.axon_hooks = mod
    except ImportError:
        pass
    try:
        mod.set_axon_ntff_profile_hook(
            trn_boot._ntff_profile_via_ctypes("/opt/axon/libaxon_pjrt.so"))
    except Exception:
        return False
    return True


def kernel(x, router_w, router_b, w1, b1, w2, b2, _trace=False):
    from concourse.bass_utils import run_bass_kernel_spmd

    if _trace:
        _trace = _install_ntff_hook()

    nc = _build()
    in_maps = _shard_inputs(x, router_w, router_b, w1, b1, w2, b2)
    res = run_bass_kernel_spmd(nc, in_maps, core_ids=list(range(N_CORES)),
                               trace=_trace)
    y = np.zeros((S, C), np.float32)
    for c in range(N_CORES):
        y += res.results[c]["y_partial"]
    aux = np.float32(res.results[0]["aux"][0, 0])
    if _trace:
        kernel.last_exec_time_ns = res.exec_time_ns
        kernel.last_results = res
    return y.reshape(B, T, C), aux
